# revision 52
# baseline (speedup 1.0000x reference)
"""Distributed AnEn (analog ensemble) kNN kernel for 8 TRN2 NeuronCores.

Strategy (SPMD, one graph on all 8 cores):
  - Historical axis sharded: core m owns dissimilarity columns [1250m, 1250m+1250).
  - Per-feature d2 = q - 2ab + r computed on the PE from host-prebuilt K=5
    operands (3 window rows, q row, ones row). The feature weight
    w_f = feature_weights/std(x_h) is computed on device from the full x_h and
    applied inside the ACT sqrt as a per-partition scale:
    sqrt(d2 * w_f^2) = w_f * sqrt(d2).
  - Feature-sum tree on gpsimd+DVE produces the negated dissimilarity slab.
  - Stage 1: 3 rounds of DVE max8/max_index/match_replace -> local top-24
    (sorted values + slab positions). 24 >= any core's share of the global
    top-50 with probability 1 - ~1e-12; a host-checked validity flag triggers
    an exact numpy fallback for the impossible miss.
  - AllToAll exchanges candidate values row-sharded (64 query rows per core);
    stage 2: 7 max8 rounds over the 192 gathered candidates -> global top-50
    in rank order. Tie-breaking matches jax.lax.top_k exactly (equal values
    resolve to the lower global index).
  - Host does pure index chasing + y lookup; every ordering decision is made
    on device.
"""

import os
import numpy as np

C = 510
CP = 512
HG = 9998
HL = 1250
F = 8
W = 3
K = 50
R1 = 3          # local rounds -> 24 candidates/core
R2 = 7          # global rounds -> 56 >= 50
K1 = 8 * R1
K2 = 8 * R2
NEG_FILL = -1.0e30
Q_PAD = 30000.0  # q for padded query rows 510/511 (fp16-representable)
H_PAD = 100.0    # x_h pad value for history rows >= 9998+2 (r_pad = 30000)
KROWS = 13       # fp16 hi/lo decomposition rows per feature
N_CORES = 8
NH_TILES = 79   # 128 * 79 = 10112 padded x_h rows
NH_PAD = 128 * NH_TILES

_GRAPH = None
LAST_EXEC_TIME_NS = None


def _build_graph():
    import concourse.bass as bass
    import concourse.bacc as bacc
    import concourse.mybir as mybir
    import concourse.tile as tile

    f32 = mybir.dt.float32
    u32 = mybir.dt.uint32
    Alu = mybir.AluOpType
    Act = mybir.ActivationFunctionType

    nc = bacc.Bacc("TRN2", target_bir_lowering=False, debug=False,
                   num_devices=N_CORES)

    f16 = mybir.dt.float16
    lhs_ext = nc.declare_dram_parameter("lhs", [F, KROWS, CP], f16, False)
    rhs_ext = nc.declare_dram_parameter("rhsd", [F, KROWS, HL], f16, False)
    xhf_ext = nc.declare_dram_parameter("xhf", [F, NH_PAD], f32, False)
    fw_ext = nc.declare_dram_parameter("fw", [1, F], f32, False)
    pos_ext = nc.declare_dram_parameter("pos_local", [CP, K1], u32, True)
    cand_ext = nc.declare_dram_parameter("cand", [64, N_CORES * K1], f32, True)
    gpos_ext = nc.declare_dram_parameter("gpos", [64, K2], u32, True)
    gval_ext = nc.declare_dram_parameter("gval", [64, K2], f32, True)

    a2a_in = nc.dram_tensor("a2a_in", [N_CORES, 64, K1], f32)
    a2a_out = nc.dram_tensor("a2a_out", [N_CORES, 64, K1], f32)
    w2_bounce = nc.dram_tensor("w2_bounce", [F], f32)

    HCH = [(0, 512), (512, 512), (1024, 226)]  # h chunks within the 1250 slab

    def dram_ap(handle, offset, ap):
        full = handle[:]
        return bass.AP(tensor=full.tensor, offset=offset, ap=ap)

    with tile.TileContext(nc) as tc:
        with (
            tc.tile_pool(name="singles", bufs=1) as singles,
            tc.tile_pool(name="work", bufs=3) as work,
        ):
            # ---------------- stds from full x_h ----------------
            # X1[p, f, j] = padded x_h[79 p + j, f] (zero pads beyond 10000)
            X1 = singles.tile([128, F, NH_TILES], f32)
            nc.scalar.dma_start(
                out=X1,
                in_=dram_ap(xhf_ext, 0,
                            [[NH_TILES, 128], [NH_PAD, F], [1, NH_TILES]]),
            )
            XQ = singles.tile([128, F, NH_TILES], f32)
            nc.vector.tensor_tensor(out=XQ, in0=X1, in1=X1, op=Alu.mult)

            ones128 = singles.tile([128, 1], f32)
            nc.vector.memset(ones128, 1.0)

            with tc.tile_pool(name="psum1", bufs=1, space="PSUM") as psum1_pool:
                ps_sum = psum1_pool.tile([1, NH_TILES * F], f32, tag="stats")
                SM = singles.tile([1, NH_TILES * F], f32)
                SM2 = singles.tile([1, NH_TILES * F], f32)
                x1flat = X1[:].rearrange("p f g -> p (f g)")
                xqflat = XQ[:].rearrange("p f g -> p (f g)")
                nc.tensor.matmul(out=ps_sum[:, 0:512], lhsT=ones128,
                                 rhs=x1flat[:, 0:512], start=True, stop=True)
                nc.tensor.matmul(out=ps_sum[:, 512:632], lhsT=ones128,
                                 rhs=x1flat[:, 512:632], start=True, stop=True)
                nc.vector.tensor_copy(SM, ps_sum)
                nc.tensor.matmul(out=ps_sum[:, 0:512], lhsT=ones128,
                                 rhs=xqflat[:, 0:512], start=True, stop=True)
                nc.tensor.matmul(out=ps_sum[:, 512:632], lhsT=ones128,
                                 rhs=xqflat[:, 512:632], start=True, stop=True)
                nc.vector.tensor_copy(SM2, ps_sum)

            # reduce the 79 per-chunk sums per feature with strided DVE adds
            def tree_reduce(t):
                v = t[:].rearrange("o (f g) -> o f g", f=F)
                n = NH_TILES
                while n > 1:
                    h = n // 2
                    nc.vector.tensor_tensor(out=v[:, :, 0:h], in0=v[:, :, 0:h],
                                            in1=v[:, :, h:2 * h], op=Alu.add)
                    if n % 2:
                        nc.vector.tensor_tensor(
                            out=v[:, :, 0:1], in0=v[:, :, 0:1],
                            in1=v[:, :, n - 1:n], op=Alu.add)
                    n = h
                return v[:, :, 0:1]

            Ssum = tree_reduce(SM).rearrange("o f x -> o (f x)")
            S2sum = tree_reduce(SM2).rearrange("o f x -> o (f x)")

            # mean/var/std/w^2 on [1, F]
            stats = singles.tile([1, 8 * F], f32)
            mu = stats[:, 0:F]
            muS = stats[:, F:2 * F]
            var = stats[:, 2 * F:3 * F]
            std0 = stats[:, 3 * F:4 * F]
            rs0 = stats[:, 4 * F:5 * F]
            wv = stats[:, 5 * F:6 * F]
            tmp = stats[:, 6 * F:7 * F]
            tmp2 = stats[:, 7 * F:8 * F]
            nc.vector.tensor_scalar(out=mu, in0=Ssum, scalar1=1.0 / 10000.0,
                                    scalar2=None, op0=Alu.mult)
            nc.vector.tensor_tensor(out=muS, in0=mu, in1=Ssum, op=Alu.mult)
            nc.vector.tensor_tensor(out=var, in0=S2sum, in1=muS, op=Alu.subtract)
            nc.vector.tensor_scalar(out=var, in0=var, scalar1=1.0 / 10000.0,
                                    scalar2=None, op0=Alu.mult)
            nc.scalar.activation(out=std0, in_=var, func=Act.Sqrt)
            # one Newton step: std = max(0.5*(std0 + var/std0), 1e-8)
            nc.vector.reciprocal(out=rs0, in_=std0)
            nc.vector.tensor_tensor(out=tmp, in0=var, in1=rs0, op=Alu.mult)
            nc.vector.tensor_tensor(out=tmp, in0=tmp, in1=std0, op=Alu.add)
            nc.vector.tensor_scalar(out=tmp, in0=tmp, scalar1=0.5, scalar2=1e-8,
                                    op0=Alu.mult, op1=Alu.max)
            # w = fw / std ; w2 = w*w
            FWt = singles.tile([1, F], f32)
            nc.sync.dma_start(out=FWt, in_=fw_ext[:])
            nc.vector.reciprocal(out=tmp2, in_=tmp)
            nc.vector.tensor_tensor(out=wv, in0=FWt, in1=tmp2, op=Alu.mult)
            nc.vector.tensor_tensor(out=tmp2, in0=wv, in1=wv, op=Alu.mult)
            nc.sync.dma_start(out=w2_bounce[:], in_=tmp2)
            w2bc = []
            for f in range(F):
                t = singles.tile([128, 1], f32, tag=f"w2bc{f}")
                w2bc.append(t)
                nc.gpsimd.dma_start(out=t,
                                    in_=dram_ap(w2_bounce, f, [[0, 128], [1, 1]]))

            # ---------------- matmul operands (host-prebuilt fp16) -----------
            # triggers split across SP and ACT HWDGE queues: the trigger
            # instruction occupies its engine ~0.7us, so one engine would
            # serialize the whole prologue
            lhsT = []
            rhs = []
            for f in range(F):
                lt = singles.tile([KROWS, CP], f16, tag=f"lhsT{f}")
                rh = singles.tile([KROWS, HL], f16, tag=f"rhs{f}")
                lhsT.append(lt)
                rhs.append(rh)
                nc.scalar.dma_start(out=lt, in_=lhs_ext[f])
                nc.sync.dma_start(out=rh, in_=rhs_ext[f])

            # ---------------- main dissimilarity + f-sum + stage 1 ----------
            psum_pool = tc.alloc_tile_pool(name="psum", bufs=2, space="PSUM")
            NEG_A = singles.tile([128, 4 * HL], f32)
            NEG_B = singles.tile([128, 4 * HL], f32)
            NEG_C = singles.tile([128, 4 * HL], f32)
            DUMP = singles.tile([128, HL], f32)
            MX = singles.tile([128, 4, K1], f32)
            PX = singles.tile([128, 4, K1], u32)
            deferred_idx = []
            for cc in range(4):
                for (h0, hn) in HCH:
                    S = work.tile([128, F, 512], f32, tag="S")
                    for fg in range(2):
                        pt = psum_pool.tile([128, 4, 512], f32, tag="d2")
                        for fi in range(4):
                            f = fg * 4 + fi
                            nc.tensor.matmul(
                                out=pt[:, fi, 0:hn],
                                lhsT=lhsT[f][:, cc * 128:(cc + 1) * 128],
                                rhs=rhs[f][:, h0:h0 + hn],
                                start=True, stop=True,
                            )
                        for fi in range(4):
                            f = fg * 4 + fi
                            nc.scalar.activation(out=S[:, f, 0:hn],
                                                 in_=pt[:, fi, 0:hn],
                                                 func=Act.Sqrt, scale=w2bc[f])
                    nc.gpsimd.tensor_tensor(out=S[:, 0:4, 0:hn], in0=S[:, 0:4, 0:hn],
                                            in1=S[:, 4:8, 0:hn], op=Alu.add)
                    nc.vector.tensor_tensor(out=S[:, 0:2, 0:hn], in0=S[:, 0:2, 0:hn],
                                            in1=S[:, 2:4, 0:hn], op=Alu.add)
                    nc.vector.tensor_tensor(out=S[:, 0, 0:hn], in0=S[:, 0, 0:hn],
                                            in1=S[:, 1, 0:hn], op=Alu.add)
                    nc.vector.tensor_scalar(
                        out=NEG_A[:, cc * HL + h0: cc * HL + h0 + hn],
                        in0=S[:, 0, 0:hn], scalar1=-1.0, scalar2=None, op0=Alu.mult)

                # stage-1 rounds: max + match_replace inline; max_index deferred
                # into the AllToAll window (A -> B -> C -> DUMP keeps each
                # round's input buffer intact for its deferred max_index)
                bufs = [NEG_A[:, cc * HL:(cc + 1) * HL],
                        NEG_B[:, cc * HL:(cc + 1) * HL],
                        NEG_C[:, cc * HL:(cc + 1) * HL],
                        DUMP[:]]
                for r in range(R1):
                    cur = bufs[r]
                    nxt = bufs[r + 1]
                    mx = MX[:, cc, 8 * r:8 * r + 8]
                    nc.vector.max(out=mx, in_=cur)
                    deferred_idx.append((PX[:, cc, 8 * r:8 * r + 8], mx, cur))
                    if r < R1 - 1:  # last round's replace output is never read
                        nc.vector.match_replace(out=nxt, in_to_replace=mx,
                                                in_values=cur, imm_value=NEG_FILL)
            psum_pool.release()

            # ---------------- A2A (2 packing DMAs: one per 64-row half) -----
            for h in range(2):
                nc.sync.dma_start(
                    out=dram_ap(a2a_in, h * 64 * K1,
                                [[K1, 64], [2 * 64 * K1, 4], [1, K1]]),
                    in_=MX[64 * h:64 * h + 64, :, :],
                )
            nc.gpsimd.collective_compute(
                "AllToAll",
                mybir.AluOpType.bypass,
                replica_groups=[list(range(N_CORES))],
                ins=[a2a_in[:]],
                outs=[a2a_out[:]],
            )
            # deferred local max_index passes overlap the collective
            for px, mx, cur in deferred_idx:
                nc.vector.max_index(out=px, in_max=mx, in_values=cur)
            CANDA = singles.tile([64, N_CORES * K1], f32)
            CANDB = singles.tile([64, N_CORES * K1], f32)
            nc.sync.dma_start(
                out=CANDA,
                in_=dram_ap(a2a_out, 0, [[K1, 64], [64 * K1, N_CORES], [1, K1]]),
            )
            # ship original candidates before the rounds clobber CANDA
            nc.scalar.dma_start(out=cand_ext[:], in_=CANDA[:])

            # ---------------- stage 2 ----------------
            GV = singles.tile([64, K2], f32)
            GP = singles.tile([64, K2], u32)
            bufs2 = [CANDA[:], CANDB[:]]
            for r in range(R2):
                cur = bufs2[r % 2]
                nxt = bufs2[(r + 1) % 2]
                gv = GV[:, 8 * r:8 * r + 8]
                nc.vector.max(out=gv, in_=cur)
                nc.vector.max_index(out=GP[:, 8 * r:8 * r + 8], in_max=gv, in_values=cur)
                if r < R2 - 1:
                    nc.vector.match_replace(out=nxt, in_to_replace=gv, in_values=cur,
                                            imm_value=NEG_FILL)

            # ---------------- outputs ----------------
            nc.sync.dma_start(
                out=dram_ap(pos_ext, 0, [[K1, 128], [128 * K1, 4], [1, K1]]),
                in_=PX[:],
            )
            nc.sync.dma_start(out=gpos_ext[:], in_=GP[:])
            nc.sync.dma_start(out=gval_ext[:], in_=GV[:])

    nc.compile()
    return nc


def _split16(x):
    """Exact fp32 = hi + lo decomposition into two fp16 parts (to ~2^-24)."""
    h = x.astype(np.float16)
    l = (x - h.astype(np.float32)).astype(np.float16)
    return h, l


def _prepare_inputs(x_t, x_h, y_h, feature_weights):
    xtT = x_t.T.astype(np.float32)                                # (8, 512)
    xhT = x_h.T.astype(np.float32)                                # (8, 10000)
    fw = feature_weights.reshape(1, F).astype(np.float32)

    xhf = np.zeros((F, NH_PAD), np.float32)
    xhf[:, :10000] = xhT

    # K rows (lhs, rhs) pairs per feature:
    #  0-2: (-2a_hi[w], b_hi[w])   3-5: (-2a_hi[w], b_lo[w])
    #  6-8: (-2a_lo[w], b_hi[w])   9: (q_hi, 1)  10: (q_lo, 1)
    #  11: (1, r_hi)  12: (1, r_lo)
    a = np.zeros((F, W, CP), np.float32)
    for w in range(W):
        a[:, w, :CP - w] = xtT[:, w:CP]
    q = ((a[:, 0] * a[:, 0] + a[:, 1] * a[:, 1]).astype(np.float32)
         + a[:, 2] * a[:, 2]).astype(np.float32)
    q[:, C:] = Q_PAD
    m2a = (-2.0 * a).astype(np.float32)
    m2a_h, m2a_l = _split16(m2a)
    q_h, q_l = _split16(q)
    lhs = np.zeros((F, KROWS, CP), np.float16)
    lhs[:, 0:W, :] = m2a_h
    lhs[:, W:2 * W, :] = m2a_h
    lhs[:, 2 * W:3 * W, :] = m2a_l
    lhs[:, 9, :] = q_h
    lhs[:, 10, :] = q_l
    lhs[:, 11:13, :] = 1.0

    in_maps = []
    for m in range(N_CORES):
        h0 = m * HL
        b = np.full((F, W, HL), H_PAD, np.float32)
        for w in range(W):
            n = max(0, min(HL, 10000 - (h0 + w)))
            b[:, w, :n] = xhT[:, h0 + w:h0 + w + n]
        r = ((b[:, 0] * b[:, 0] + b[:, 1] * b[:, 1]).astype(np.float32)
             + b[:, 2] * b[:, 2]).astype(np.float32)
        b_h, b_l = _split16(b)
        r_h, r_l = _split16(r)
        rhsd = np.zeros((F, KROWS, HL), np.float16)
        rhsd[:, 0:W, :] = b_h
        rhsd[:, W:2 * W, :] = b_l
        rhsd[:, 2 * W:3 * W, :] = b_h
        rhsd[:, 9:11, :] = 1.0
        rhsd[:, 11, :] = r_h
        rhsd[:, 12, :] = r_l
        in_maps.append({
            "lhs": lhs,
            "rhsd": rhsd,
            "xhf": xhf,
            "fw": fw,
        })
    return in_maps


def _assemble(results, y_h):
    """Host-side index chasing + y lookup. Returns (out, ok)."""
    yfull = y_h[:, 0].astype(np.float32)
    out = np.zeros((C, K), np.float32)
    ok = True
    pos_local = [results[m]["pos_local"] for m in range(N_CORES)]
    for owner in range(N_CORES):
        gp = results[owner]["gpos"]          # (64, 56) uint32
        gv = results[owner]["gval"]          # (64, 56)
        cand = results[owner]["cand"]        # (64, 192)
        nrows = 64 if owner < N_CORES - 1 else 64 - (CP - C)
        # validity: no core's 24th-best may beat the global 50th best
        last = cand[:nrows, K1 - 1::K1]
        if (last.max(axis=1) >= gv[:nrows, K - 1]).any():
            ok = False
        for j in range(nrows):
            c = owner * 64 + j
            p = gp[j, :K].astype(np.int64)
            m = p // K1
            t = p % K1
            lp = np.array([pos_local[mm][c, tt] for mm, tt in zip(m, t)], np.int64)
            out[c] = yfull[1 + HL * m + lp]
    return out, ok


def _reference_fallback(x_t, x_h, y_h, feature_weights):
    """Exact numpy replica of the reference (used only if the validity flag
    trips, which has probability ~1e-12)."""
    stds = np.maximum(np.std(x_h.astype(np.float32), axis=0), 1e-8)
    wn = (feature_weights.astype(np.float32) / stds).astype(np.float32)
    n_c = x_t.shape[0] - W + 1
    n_h = x_h.shape[0] - W + 1
    idx_c = np.arange(n_c)[:, None] + np.arange(W)[None, :]
    idx_h = np.arange(n_h)[:, None] + np.arange(W)[None, :]
    cw = x_t[idx_c]
    hw = x_h[idx_h]
    D = np.zeros((n_c, n_h), np.float32)
    for f in range(F):
        d2 = np.zeros((n_c, n_h), np.float32)
        for wi in range(W):
            diff = (cw[:, None, wi, f] - hw[None, :, wi, f]).astype(np.float32)
            d2 = (d2 + diff * diff).astype(np.float32)
        D = (D + wn[f] * np.sqrt(d2)).astype(np.float32)
    yv = y_h[W // 2:-(W // 2), 0]
    out = np.empty((n_c, K), np.float32)
    for c in range(n_c):
        order = np.lexsort((np.arange(n_h), D[c]))[:K]
        out[c] = yv[order]
    return out


def _ensure_axon_hooks():
    """The agent image's antenv lacks axon_hooks; inject an equivalent module
    and register the ctypes NTFF profile hook so trace=True works."""
    import sys
    import types
    import ctypes
    import contextlib

    try:
        from antenv.axon_hooks import get_axon_ntff_profile_hook  # noqa: F401
        return True
    except ImportError:
        pass
    try:
        import antenv
    except ImportError:
        return False
    so_path = "/opt/axon/libaxon_pjrt.so"
    if not os.path.exists(so_path):
        return False

    mod = types.ModuleType("antenv.axon_hooks")
    holder = {"hook": None}
    mod.set_axon_ntff_profile_hook = lambda h: holder.__setitem__("hook", h)
    mod.get_axon_ntff_profile_hook = lambda: holder["hook"]
    sys.modules["antenv.axon_hooks"] = mod
    antenv.axon_hooks = mod

    lib = ctypes.CDLL(so_path)
    if not hasattr(lib, "axon_start_nrt_profile"):
        return False
    lib.axon_start_nrt_profile.argtypes = [ctypes.POINTER(ctypes.c_int64),
                                           ctypes.c_size_t]
    lib.axon_start_nrt_profile.restype = ctypes.c_int64
    lib.axon_stop_nrt_profile.argtypes = [ctypes.c_char_p]
    lib.axon_stop_nrt_profile.restype = ctypes.c_int64

    @contextlib.contextmanager
    def _hook(output_dir, device_ids):
        import jax
        jax.devices()
        if device_ids:
            ids = (ctypes.c_int64 * len(device_ids))(*device_ids)
            rc = lib.axon_start_nrt_profile(ids, len(device_ids))
        else:
            rc = lib.axon_start_nrt_profile(None, 0)
        if rc != 0:
            raise RuntimeError(f"axon_start_nrt_profile rc={rc}")
        try:
            yield
        finally:
            n = lib.axon_stop_nrt_profile(str(output_dir).encode())
            if n <= 0:
                print(f"profile: {n} file(s) written to {output_dir}")

    mod.set_axon_ntff_profile_hook(_hook)
    return True


def kernel(x_t, x_h, y_h, feature_weights):
    global _GRAPH, LAST_EXEC_TIME_NS
    from concourse.bass_utils import run_bass_kernel_spmd

    x_t = np.asarray(x_t, np.float32)
    x_h = np.asarray(x_h, np.float32)
    y_h = np.asarray(y_h, np.float32)
    feature_weights = np.asarray(feature_weights, np.float32)

    if _GRAPH is None:
        _GRAPH = _build_graph()
    in_maps = _prepare_inputs(x_t, x_h, y_h, feature_weights)
    trace = bool(int(os.environ.get("KERNEL_TRACE", "0")))
    if trace and not _ensure_axon_hooks():
        trace = False
    res = run_bass_kernel_spmd(_GRAPH, in_maps, core_ids=list(range(N_CORES)),
                               trace=trace)
    LAST_EXEC_TIME_NS = res.exec_time_ns
    out, ok = _assemble(res.results, y_h)
    if not ok:
        out = _reference_fallback(x_t, x_h, y_h, feature_weights)
    return out


# revision 53
# speedup vs baseline: 1.1037x; 1.1037x over previous
"""Distributed AnEn (analog ensemble) kNN kernel for 8 TRN2 NeuronCores.

Strategy (SPMD, one graph on all 8 cores):
  - Historical axis sharded: core m owns dissimilarity columns [1250m, 1250m+1250).
  - Per-feature d2 = q - 2ab + r computed on the PE from host-prebuilt K=5
    operands (3 window rows, q row, ones row). The feature weight
    w_f = feature_weights/std(x_h) is computed on device from the full x_h and
    applied inside the ACT sqrt as a per-partition scale:
    sqrt(d2 * w_f^2) = w_f * sqrt(d2).
  - Feature-sum tree on gpsimd+DVE produces the negated dissimilarity slab.
  - Stage 1: 3 rounds of DVE max8/max_index/match_replace -> local top-24
    (sorted values + slab positions). 24 >= any core's share of the global
    top-50 with probability 1 - ~1e-12; a host-checked validity flag triggers
    an exact numpy fallback for the impossible miss.
  - AllToAll exchanges candidate values row-sharded (64 query rows per core);
    stage 2: 7 max8 rounds over the 192 gathered candidates -> global top-50
    in rank order. Tie-breaking matches jax.lax.top_k exactly (equal values
    resolve to the lower global index).
  - Host does pure index chasing + y lookup; every ordering decision is made
    on device.
"""

import os
import numpy as np

C = 510
CP = 512
HG = 9998
HL = 1250
F = 8
W = 3
K = 50
R1 = 3          # local rounds -> 24 candidates/core
R2 = 7          # global rounds -> 56 >= 50
K1 = 8 * R1
K2 = 8 * R2
NEG_FILL = -1.0e30
Q_PAD = 30000.0  # q for padded query rows 510/511 (fp16-representable)
H_PAD = 100.0    # x_h pad value for history rows >= 9998+2 (r_pad = 30000)
KROWS = 13       # fp16 hi/lo decomposition rows per feature
N_CORES = 8
NH_TILES = 79   # 128 * 79 = 10112 padded x_h rows
NH_PAD = 128 * NH_TILES

_GRAPH = None
LAST_EXEC_TIME_NS = None


def _build_graph():
    import concourse.bass as bass
    import concourse.bacc as bacc
    import concourse.mybir as mybir
    import concourse.tile as tile

    f32 = mybir.dt.float32
    u32 = mybir.dt.uint32
    Alu = mybir.AluOpType
    Act = mybir.ActivationFunctionType

    nc = bacc.Bacc("TRN2", target_bir_lowering=False, debug=False,
                   num_devices=N_CORES)

    f16 = mybir.dt.float16
    lhs_ext = nc.declare_dram_parameter("lhs", [F, KROWS, CP], f16, False)
    rhs_ext = nc.declare_dram_parameter("rhsd", [F, KROWS, HL], f16, False)
    xhf_ext = nc.declare_dram_parameter("xhf", [F, NH_PAD], f32, False)
    fw_ext = nc.declare_dram_parameter("fw", [1, F], f32, False)
    pos_ext = nc.declare_dram_parameter("pos_local", [CP, K1], u32, True)
    cand_ext = nc.declare_dram_parameter("cand", [64, N_CORES * K1], f32, True)
    gpos_ext = nc.declare_dram_parameter("gpos", [64, K2], u32, True)
    gval_ext = nc.declare_dram_parameter("gval", [64, K2], f32, True)

    a2a_in = nc.dram_tensor("a2a_in", [N_CORES, 64, K1], f32)
    a2a_out = nc.dram_tensor("a2a_out", [N_CORES, 64, K1], f32)
    w2_bounce = nc.dram_tensor("w2_bounce", [F], f32)

    HCH = [(0, 512), (512, 512), (1024, 226)]  # h chunks within the 1250 slab

    def dram_ap(handle, offset, ap):
        full = handle[:]
        return bass.AP(tensor=full.tensor, offset=offset, ap=ap)

    with tile.TileContext(nc) as tc:
        with (
            tc.tile_pool(name="singles", bufs=1) as singles,
            tc.tile_pool(name="work", bufs=3) as work,
        ):
            # ---------------- stds from full x_h ----------------
            # X1[p, f, j] = padded x_h[79 p + j, f] (zero pads beyond 10000)
            X1 = singles.tile([128, F, NH_TILES], f32)
            nc.sync.dma_start(
                out=X1,
                in_=dram_ap(xhf_ext, 0,
                            [[NH_TILES, 128], [NH_PAD, F], [1, NH_TILES]]),
            )
            XQ = singles.tile([128, F, NH_TILES], f32)
            nc.vector.tensor_tensor(out=XQ, in0=X1, in1=X1, op=Alu.mult)

            ones128 = singles.tile([128, 1], f32)
            nc.vector.memset(ones128, 1.0)

            with tc.tile_pool(name="psum1", bufs=1, space="PSUM") as psum1_pool:
                ps_sum = psum1_pool.tile([1, NH_TILES * F], f32, tag="stats")
                SM = singles.tile([1, NH_TILES * F], f32)
                SM2 = singles.tile([1, NH_TILES * F], f32)
                x1flat = X1[:].rearrange("p f g -> p (f g)")
                xqflat = XQ[:].rearrange("p f g -> p (f g)")
                nc.tensor.matmul(out=ps_sum[:, 0:512], lhsT=ones128,
                                 rhs=x1flat[:, 0:512], start=True, stop=True)
                nc.tensor.matmul(out=ps_sum[:, 512:632], lhsT=ones128,
                                 rhs=x1flat[:, 512:632], start=True, stop=True)
                nc.vector.tensor_copy(SM, ps_sum)
                nc.tensor.matmul(out=ps_sum[:, 0:512], lhsT=ones128,
                                 rhs=xqflat[:, 0:512], start=True, stop=True)
                nc.tensor.matmul(out=ps_sum[:, 512:632], lhsT=ones128,
                                 rhs=xqflat[:, 512:632], start=True, stop=True)
                nc.vector.tensor_copy(SM2, ps_sum)

            # reduce the 79 per-chunk sums per feature with strided DVE adds
            def tree_reduce(t):
                v = t[:].rearrange("o (f g) -> o f g", f=F)
                n = NH_TILES
                while n > 1:
                    h = n // 2
                    nc.vector.tensor_tensor(out=v[:, :, 0:h], in0=v[:, :, 0:h],
                                            in1=v[:, :, h:2 * h], op=Alu.add)
                    if n % 2:
                        nc.vector.tensor_tensor(
                            out=v[:, :, 0:1], in0=v[:, :, 0:1],
                            in1=v[:, :, n - 1:n], op=Alu.add)
                    n = h
                return v[:, :, 0:1]

            Ssum = tree_reduce(SM).rearrange("o f x -> o (f x)")
            S2sum = tree_reduce(SM2).rearrange("o f x -> o (f x)")

            # mean/var/std/w^2 on [1, F]
            stats = singles.tile([1, 8 * F], f32)
            mu = stats[:, 0:F]
            muS = stats[:, F:2 * F]
            var = stats[:, 2 * F:3 * F]
            std0 = stats[:, 3 * F:4 * F]
            rs0 = stats[:, 4 * F:5 * F]
            wv = stats[:, 5 * F:6 * F]
            tmp = stats[:, 6 * F:7 * F]
            tmp2 = stats[:, 7 * F:8 * F]
            nc.vector.tensor_scalar(out=mu, in0=Ssum, scalar1=1.0 / 10000.0,
                                    scalar2=None, op0=Alu.mult)
            nc.vector.tensor_tensor(out=muS, in0=mu, in1=Ssum, op=Alu.mult)
            nc.vector.tensor_tensor(out=var, in0=S2sum, in1=muS, op=Alu.subtract)
            nc.vector.tensor_scalar(out=var, in0=var, scalar1=1.0 / 10000.0,
                                    scalar2=None, op0=Alu.mult)
            nc.scalar.activation(out=std0, in_=var, func=Act.Sqrt)
            # one Newton step: std = max(0.5*(std0 + var/std0), 1e-8)
            nc.vector.reciprocal(out=rs0, in_=std0)
            nc.vector.tensor_tensor(out=tmp, in0=var, in1=rs0, op=Alu.mult)
            nc.vector.tensor_tensor(out=tmp, in0=tmp, in1=std0, op=Alu.add)
            nc.vector.tensor_scalar(out=tmp, in0=tmp, scalar1=0.5, scalar2=1e-8,
                                    op0=Alu.mult, op1=Alu.max)
            # w = fw / std ; w2 = w*w
            FWt = singles.tile([1, F], f32)
            nc.sync.dma_start(out=FWt, in_=fw_ext[:])
            nc.vector.reciprocal(out=tmp2, in_=tmp)
            nc.vector.tensor_tensor(out=wv, in0=FWt, in1=tmp2, op=Alu.mult)
            nc.vector.tensor_tensor(out=tmp2, in0=wv, in1=wv, op=Alu.mult)
            nc.sync.dma_start(out=w2_bounce[:], in_=tmp2)
            w2bc = []
            for f in range(F):
                t = singles.tile([128, 1], f32, tag=f"w2bc{f}")
                w2bc.append(t)
                nc.sync.dma_start(out=t,
                                  in_=dram_ap(w2_bounce, f, [[0, 128], [1, 1]]))

            # ---------------- matmul operands (host-prebuilt fp16) -----------
            # triggers split across SP and ACT HWDGE queues: the trigger
            # instruction occupies its engine ~0.7us, so one engine would
            # serialize the whole prologue
            lhsT = []
            rhs = []
            for f in range(F):
                lt = singles.tile([KROWS, CP], f16, tag=f"lhsT{f}")
                rh = singles.tile([KROWS, HL], f16, tag=f"rhs{f}")
                lhsT.append(lt)
                rhs.append(rh)
                nc.sync.dma_start(out=lt, in_=lhs_ext[f])
                nc.sync.dma_start(out=rh, in_=rhs_ext[f])

            # ---------------- main dissimilarity + f-sum + stage 1 ----------
            psum_pool = tc.alloc_tile_pool(name="psum", bufs=2, space="PSUM")
            NEG_A = singles.tile([128, 4 * HL], f32)
            NEG_B = singles.tile([128, 4 * HL], f32)
            NEG_C = singles.tile([128, 4 * HL], f32)
            DUMP = singles.tile([128, HL], f32)
            MX = singles.tile([128, 4, K1], f32)
            PX = singles.tile([128, 4, K1], u32)
            deferred_idx = []
            for cc in range(4):
                for (h0, hn) in HCH:
                    S = work.tile([128, F, 512], f32, tag="S")
                    for fg in range(2):
                        pt = psum_pool.tile([128, 4, 512], f32, tag="d2")
                        for fi in range(4):
                            f = fg * 4 + fi
                            nc.tensor.matmul(
                                out=pt[:, fi, 0:hn],
                                lhsT=lhsT[f][:, cc * 128:(cc + 1) * 128],
                                rhs=rhs[f][:, h0:h0 + hn],
                                start=True, stop=True,
                            )
                        for fi in range(4):
                            f = fg * 4 + fi
                            nc.scalar.activation(out=S[:, f, 0:hn],
                                                 in_=pt[:, fi, 0:hn],
                                                 func=Act.Sqrt, scale=w2bc[f])
                    nc.gpsimd.tensor_tensor(out=S[:, 0:4, 0:hn], in0=S[:, 0:4, 0:hn],
                                            in1=S[:, 4:8, 0:hn], op=Alu.add)
                    nc.vector.tensor_tensor(out=S[:, 0:2, 0:hn], in0=S[:, 0:2, 0:hn],
                                            in1=S[:, 2:4, 0:hn], op=Alu.add)
                    nc.vector.tensor_tensor(out=S[:, 0, 0:hn], in0=S[:, 0, 0:hn],
                                            in1=S[:, 1, 0:hn], op=Alu.add)
                    nc.vector.tensor_scalar(
                        out=NEG_A[:, cc * HL + h0: cc * HL + h0 + hn],
                        in0=S[:, 0, 0:hn], scalar1=-1.0, scalar2=None, op0=Alu.mult)

                # stage-1 rounds: max + match_replace inline; max_index deferred
                # into the AllToAll window (A -> B -> C -> DUMP keeps each
                # round's input buffer intact for its deferred max_index)
                bufs = [NEG_A[:, cc * HL:(cc + 1) * HL],
                        NEG_B[:, cc * HL:(cc + 1) * HL],
                        NEG_C[:, cc * HL:(cc + 1) * HL],
                        DUMP[:]]
                for r in range(R1):
                    cur = bufs[r]
                    nxt = bufs[r + 1]
                    mx = MX[:, cc, 8 * r:8 * r + 8]
                    nc.vector.max(out=mx, in_=cur)
                    deferred_idx.append((PX[:, cc, 8 * r:8 * r + 8], mx, cur))
                    if r < R1 - 1:  # last round's replace output is never read
                        nc.vector.match_replace(out=nxt, in_to_replace=mx,
                                                in_values=cur, imm_value=NEG_FILL)
            psum_pool.release()

            # ---------------- A2A (2 packing DMAs: one per 64-row half) -----
            for h in range(2):
                nc.sync.dma_start(
                    out=dram_ap(a2a_in, h * 64 * K1,
                                [[K1, 64], [2 * 64 * K1, 4], [1, K1]]),
                    in_=MX[64 * h:64 * h + 64, :, :],
                )
            nc.gpsimd.collective_compute(
                "AllToAll",
                mybir.AluOpType.bypass,
                replica_groups=[list(range(N_CORES))],
                ins=[a2a_in[:]],
                outs=[a2a_out[:]],
            )
            # deferred local max_index passes overlap the collective
            for px, mx, cur in deferred_idx:
                nc.vector.max_index(out=px, in_max=mx, in_values=cur)
            CANDA = singles.tile([64, N_CORES * K1], f32)
            CANDB = singles.tile([64, N_CORES * K1], f32)
            nc.sync.dma_start(
                out=CANDA,
                in_=dram_ap(a2a_out, 0, [[K1, 64], [64 * K1, N_CORES], [1, K1]]),
            )
            # ship original candidates before the rounds clobber CANDA
            nc.sync.dma_start(out=cand_ext[:], in_=CANDA[:])

            # ---------------- stage 2 ----------------
            GV = singles.tile([64, K2], f32)
            GP = singles.tile([64, K2], u32)
            bufs2 = [CANDA[:], CANDB[:]]
            for r in range(R2):
                cur = bufs2[r % 2]
                nxt = bufs2[(r + 1) % 2]
                gv = GV[:, 8 * r:8 * r + 8]
                nc.vector.max(out=gv, in_=cur)
                nc.vector.max_index(out=GP[:, 8 * r:8 * r + 8], in_max=gv, in_values=cur)
                if r < R2 - 1:
                    nc.vector.match_replace(out=nxt, in_to_replace=gv, in_values=cur,
                                            imm_value=NEG_FILL)

            # ---------------- outputs ----------------
            nc.sync.dma_start(
                out=dram_ap(pos_ext, 0, [[K1, 128], [128 * K1, 4], [1, K1]]),
                in_=PX[:],
            )
            nc.sync.dma_start(out=gpos_ext[:], in_=GP[:])
            nc.sync.dma_start(out=gval_ext[:], in_=GV[:])

    nc.compile()
    return nc


def _split16(x):
    """Exact fp32 = hi + lo decomposition into two fp16 parts (to ~2^-24)."""
    h = x.astype(np.float16)
    l = (x - h.astype(np.float32)).astype(np.float16)
    return h, l


def _prepare_inputs(x_t, x_h, y_h, feature_weights):
    xtT = x_t.T.astype(np.float32)                                # (8, 512)
    xhT = x_h.T.astype(np.float32)                                # (8, 10000)
    fw = feature_weights.reshape(1, F).astype(np.float32)

    xhf = np.zeros((F, NH_PAD), np.float32)
    xhf[:, :10000] = xhT

    # K rows (lhs, rhs) pairs per feature:
    #  0-2: (-2a_hi[w], b_hi[w])   3-5: (-2a_hi[w], b_lo[w])
    #  6-8: (-2a_lo[w], b_hi[w])   9: (q_hi, 1)  10: (q_lo, 1)
    #  11: (1, r_hi)  12: (1, r_lo)
    a = np.zeros((F, W, CP), np.float32)
    for w in range(W):
        a[:, w, :CP - w] = xtT[:, w:CP]
    q = ((a[:, 0] * a[:, 0] + a[:, 1] * a[:, 1]).astype(np.float32)
         + a[:, 2] * a[:, 2]).astype(np.float32)
    q[:, C:] = Q_PAD
    m2a = (-2.0 * a).astype(np.float32)
    m2a_h, m2a_l = _split16(m2a)
    q_h, q_l = _split16(q)
    lhs = np.zeros((F, KROWS, CP), np.float16)
    lhs[:, 0:W, :] = m2a_h
    lhs[:, W:2 * W, :] = m2a_h
    lhs[:, 2 * W:3 * W, :] = m2a_l
    lhs[:, 9, :] = q_h
    lhs[:, 10, :] = q_l
    lhs[:, 11:13, :] = 1.0

    in_maps = []
    for m in range(N_CORES):
        h0 = m * HL
        b = np.full((F, W, HL), H_PAD, np.float32)
        for w in range(W):
            n = max(0, min(HL, 10000 - (h0 + w)))
            b[:, w, :n] = xhT[:, h0 + w:h0 + w + n]
        r = ((b[:, 0] * b[:, 0] + b[:, 1] * b[:, 1]).astype(np.float32)
             + b[:, 2] * b[:, 2]).astype(np.float32)
        b_h, b_l = _split16(b)
        r_h, r_l = _split16(r)
        rhsd = np.zeros((F, KROWS, HL), np.float16)
        rhsd[:, 0:W, :] = b_h
        rhsd[:, W:2 * W, :] = b_l
        rhsd[:, 2 * W:3 * W, :] = b_h
        rhsd[:, 9:11, :] = 1.0
        rhsd[:, 11, :] = r_h
        rhsd[:, 12, :] = r_l
        in_maps.append({
            "lhs": lhs,
            "rhsd": rhsd,
            "xhf": xhf,
            "fw": fw,
        })
    return in_maps


def _assemble(results, y_h):
    """Host-side index chasing + y lookup. Returns (out, ok)."""
    yfull = y_h[:, 0].astype(np.float32)
    out = np.zeros((C, K), np.float32)
    ok = True
    pos_local = [results[m]["pos_local"] for m in range(N_CORES)]
    for owner in range(N_CORES):
        gp = results[owner]["gpos"]          # (64, 56) uint32
        gv = results[owner]["gval"]          # (64, 56)
        cand = results[owner]["cand"]        # (64, 192)
        nrows = 64 if owner < N_CORES - 1 else 64 - (CP - C)
        # validity: no core's 24th-best may beat the global 50th best
        last = cand[:nrows, K1 - 1::K1]
        if (last.max(axis=1) >= gv[:nrows, K - 1]).any():
            ok = False
        for j in range(nrows):
            c = owner * 64 + j
            p = gp[j, :K].astype(np.int64)
            m = p // K1
            t = p % K1
            lp = np.array([pos_local[mm][c, tt] for mm, tt in zip(m, t)], np.int64)
            out[c] = yfull[1 + HL * m + lp]
    return out, ok


def _reference_fallback(x_t, x_h, y_h, feature_weights):
    """Exact numpy replica of the reference (used only if the validity flag
    trips, which has probability ~1e-12)."""
    stds = np.maximum(np.std(x_h.astype(np.float32), axis=0), 1e-8)
    wn = (feature_weights.astype(np.float32) / stds).astype(np.float32)
    n_c = x_t.shape[0] - W + 1
    n_h = x_h.shape[0] - W + 1
    idx_c = np.arange(n_c)[:, None] + np.arange(W)[None, :]
    idx_h = np.arange(n_h)[:, None] + np.arange(W)[None, :]
    cw = x_t[idx_c]
    hw = x_h[idx_h]
    D = np.zeros((n_c, n_h), np.float32)
    for f in range(F):
        d2 = np.zeros((n_c, n_h), np.float32)
        for wi in range(W):
            diff = (cw[:, None, wi, f] - hw[None, :, wi, f]).astype(np.float32)
            d2 = (d2 + diff * diff).astype(np.float32)
        D = (D + wn[f] * np.sqrt(d2)).astype(np.float32)
    yv = y_h[W // 2:-(W // 2), 0]
    out = np.empty((n_c, K), np.float32)
    for c in range(n_c):
        order = np.lexsort((np.arange(n_h), D[c]))[:K]
        out[c] = yv[order]
    return out


def _ensure_axon_hooks():
    """The agent image's antenv lacks axon_hooks; inject an equivalent module
    and register the ctypes NTFF profile hook so trace=True works."""
    import sys
    import types
    import ctypes
    import contextlib

    try:
        from antenv.axon_hooks import get_axon_ntff_profile_hook  # noqa: F401
        return True
    except ImportError:
        pass
    try:
        import antenv
    except ImportError:
        return False
    so_path = "/opt/axon/libaxon_pjrt.so"
    if not os.path.exists(so_path):
        return False

    mod = types.ModuleType("antenv.axon_hooks")
    holder = {"hook": None}
    mod.set_axon_ntff_profile_hook = lambda h: holder.__setitem__("hook", h)
    mod.get_axon_ntff_profile_hook = lambda: holder["hook"]
    sys.modules["antenv.axon_hooks"] = mod
    antenv.axon_hooks = mod

    lib = ctypes.CDLL(so_path)
    if not hasattr(lib, "axon_start_nrt_profile"):
        return False
    lib.axon_start_nrt_profile.argtypes = [ctypes.POINTER(ctypes.c_int64),
                                           ctypes.c_size_t]
    lib.axon_start_nrt_profile.restype = ctypes.c_int64
    lib.axon_stop_nrt_profile.argtypes = [ctypes.c_char_p]
    lib.axon_stop_nrt_profile.restype = ctypes.c_int64

    @contextlib.contextmanager
    def _hook(output_dir, device_ids):
        import jax
        jax.devices()
        if device_ids:
            ids = (ctypes.c_int64 * len(device_ids))(*device_ids)
            rc = lib.axon_start_nrt_profile(ids, len(device_ids))
        else:
            rc = lib.axon_start_nrt_profile(None, 0)
        if rc != 0:
            raise RuntimeError(f"axon_start_nrt_profile rc={rc}")
        try:
            yield
        finally:
            n = lib.axon_stop_nrt_profile(str(output_dir).encode())
            if n <= 0:
                print(f"profile: {n} file(s) written to {output_dir}")

    mod.set_axon_ntff_profile_hook(_hook)
    return True


def kernel(x_t, x_h, y_h, feature_weights):
    global _GRAPH, LAST_EXEC_TIME_NS
    from concourse.bass_utils import run_bass_kernel_spmd

    x_t = np.asarray(x_t, np.float32)
    x_h = np.asarray(x_h, np.float32)
    y_h = np.asarray(y_h, np.float32)
    feature_weights = np.asarray(feature_weights, np.float32)

    if _GRAPH is None:
        _GRAPH = _build_graph()
    in_maps = _prepare_inputs(x_t, x_h, y_h, feature_weights)
    trace = bool(int(os.environ.get("KERNEL_TRACE", "0")))
    if trace and not _ensure_axon_hooks():
        trace = False
    res = run_bass_kernel_spmd(_GRAPH, in_maps, core_ids=list(range(N_CORES)),
                               trace=trace)
    LAST_EXEC_TIME_NS = res.exec_time_ns
    out, ok = _assemble(res.results, y_h)
    if not ok:
        out = _reference_fallback(x_t, x_h, y_h, feature_weights)
    return out


# revision 55
# speedup vs baseline: 1.1712x; 1.0612x over previous
"""Distributed AnEn (analog ensemble) kNN kernel for 8 TRN2 NeuronCores.

Strategy (SPMD, one graph on all 8 cores):
  - Historical axis sharded: core m owns dissimilarity columns [1250m, 1250m+1250).
  - Per-feature d2 = q - 2ab + r computed on the PE from host-prebuilt K=5
    operands (3 window rows, q row, ones row). The feature weight
    w_f = feature_weights/std(x_h) is computed on device from the full x_h and
    applied inside the ACT sqrt as a per-partition scale:
    sqrt(d2 * w_f^2) = w_f * sqrt(d2).
  - Feature-sum tree on gpsimd+DVE produces the negated dissimilarity slab.
  - Stage 1: 3 rounds of DVE max8/max_index/match_replace -> local top-24
    (sorted values + slab positions). 24 >= any core's share of the global
    top-50 with probability 1 - ~1e-12; a host-checked validity flag triggers
    an exact numpy fallback for the impossible miss.
  - AllToAll exchanges candidate values row-sharded (64 query rows per core);
    stage 2: 7 max8 rounds over the 192 gathered candidates -> global top-50
    in rank order. Tie-breaking matches jax.lax.top_k exactly (equal values
    resolve to the lower global index).
  - Host does pure index chasing + y lookup; every ordering decision is made
    on device.
"""

import os
import numpy as np

C = 510
CP = 512
HG = 9998
HL = 1250
F = 8
W = 3
K = 50
R1 = 3          # local rounds -> 24 candidates/core
R2 = 7          # global rounds -> 56 >= 50
K1 = 8 * R1
K2 = 8 * R2
NEG_FILL = -1.0e30
Q_PAD = 30000.0  # q for padded query rows 510/511 (fp16-representable)
H_PAD = 100.0    # x_h pad value for history rows >= 9998+2 (r_pad = 30000)
KROWS = 13       # fp16 hi/lo decomposition rows per feature
N_CORES = 8
NH_TILES = 79   # 128 * 79 = 10112 padded x_h rows
NH_PAD = 128 * NH_TILES

_GRAPH = None
LAST_EXEC_TIME_NS = None


def _build_graph():
    import concourse.bass as bass
    import concourse.bacc as bacc
    import concourse.mybir as mybir
    import concourse.tile as tile

    f32 = mybir.dt.float32
    u32 = mybir.dt.uint32
    Alu = mybir.AluOpType
    Act = mybir.ActivationFunctionType

    nc = bacc.Bacc("TRN2", target_bir_lowering=False, debug=False,
                   num_devices=N_CORES)

    f16 = mybir.dt.float16
    lhs_ext = nc.declare_dram_parameter("lhs", [F, KROWS, CP], f16, False)
    rhs_ext = nc.declare_dram_parameter("rhsd", [F, KROWS, HL], f16, False)
    xhf_ext = nc.declare_dram_parameter("xhf", [F, NH_PAD], f32, False)
    fw_ext = nc.declare_dram_parameter("fw", [1, F], f32, False)
    pos_ext = nc.declare_dram_parameter("pos_local", [CP, K1], u32, True)
    cand_ext = nc.declare_dram_parameter("cand", [64, N_CORES * K1], f32, True)
    gpos_ext = nc.declare_dram_parameter("gpos", [64, K2], u32, True)
    gval_ext = nc.declare_dram_parameter("gval", [64, K2], f32, True)

    a2a_in = nc.dram_tensor("a2a_in", [N_CORES, 64, K1], f32)
    a2a_out = nc.dram_tensor("a2a_out", [N_CORES, 64, K1], f32)
    w2_bounce = nc.dram_tensor("w2_bounce", [F], f32)

    HCH = [(0, 512), (512, 512), (1024, 226)]  # h chunks within the 1250 slab

    def dram_ap(handle, offset, ap):
        full = handle[:]
        return bass.AP(tensor=full.tensor, offset=offset, ap=ap)

    with tile.TileContext(nc) as tc:
        with (
            tc.tile_pool(name="singles", bufs=1) as singles,
            tc.tile_pool(name="work", bufs=2) as work,
        ):
            # ---------------- stds from full x_h ----------------
            # X1[p, f, j] = padded x_h[79 p + j, f] (zero pads beyond 10000)
            X1 = singles.tile([128, F, NH_TILES], f32)
            nc.sync.dma_start(
                out=X1,
                in_=dram_ap(xhf_ext, 0,
                            [[NH_TILES, 128], [NH_PAD, F], [1, NH_TILES]]),
            )
            XQ = singles.tile([128, F, NH_TILES], f32)
            nc.vector.tensor_tensor(out=XQ, in0=X1, in1=X1, op=Alu.mult)

            ones128 = singles.tile([128, 1], f32)
            nc.vector.memset(ones128, 1.0)

            with tc.tile_pool(name="psum1", bufs=1, space="PSUM") as psum1_pool:
                ps_sum = psum1_pool.tile([1, NH_TILES * F], f32, tag="stats")
                SM = singles.tile([1, NH_TILES * F], f32)
                SM2 = singles.tile([1, NH_TILES * F], f32)
                x1flat = X1[:].rearrange("p f g -> p (f g)")
                xqflat = XQ[:].rearrange("p f g -> p (f g)")
                nc.tensor.matmul(out=ps_sum[:, 0:512], lhsT=ones128,
                                 rhs=x1flat[:, 0:512], start=True, stop=True)
                nc.tensor.matmul(out=ps_sum[:, 512:632], lhsT=ones128,
                                 rhs=x1flat[:, 512:632], start=True, stop=True)
                nc.vector.tensor_copy(SM, ps_sum)
                nc.tensor.matmul(out=ps_sum[:, 0:512], lhsT=ones128,
                                 rhs=xqflat[:, 0:512], start=True, stop=True)
                nc.tensor.matmul(out=ps_sum[:, 512:632], lhsT=ones128,
                                 rhs=xqflat[:, 512:632], start=True, stop=True)
                nc.vector.tensor_copy(SM2, ps_sum)

            # reduce the 79 per-chunk sums per feature with strided DVE adds
            def tree_reduce(t):
                v = t[:].rearrange("o (f g) -> o f g", f=F)
                n = NH_TILES
                while n > 1:
                    h = n // 2
                    nc.vector.tensor_tensor(out=v[:, :, 0:h], in0=v[:, :, 0:h],
                                            in1=v[:, :, h:2 * h], op=Alu.add)
                    if n % 2:
                        nc.vector.tensor_tensor(
                            out=v[:, :, 0:1], in0=v[:, :, 0:1],
                            in1=v[:, :, n - 1:n], op=Alu.add)
                    n = h
                return v[:, :, 0:1]

            Ssum = tree_reduce(SM).rearrange("o f x -> o (f x)")
            S2sum = tree_reduce(SM2).rearrange("o f x -> o (f x)")

            # mean/var/std/w^2 on [1, F]
            stats = singles.tile([1, 8 * F], f32)
            mu = stats[:, 0:F]
            muS = stats[:, F:2 * F]
            var = stats[:, 2 * F:3 * F]
            std0 = stats[:, 3 * F:4 * F]
            rs0 = stats[:, 4 * F:5 * F]
            wv = stats[:, 5 * F:6 * F]
            tmp = stats[:, 6 * F:7 * F]
            tmp2 = stats[:, 7 * F:8 * F]
            nc.vector.tensor_scalar(out=mu, in0=Ssum, scalar1=1.0 / 10000.0,
                                    scalar2=None, op0=Alu.mult)
            nc.vector.tensor_tensor(out=muS, in0=mu, in1=Ssum, op=Alu.mult)
            nc.vector.tensor_tensor(out=var, in0=S2sum, in1=muS, op=Alu.subtract)
            nc.vector.tensor_scalar(out=var, in0=var, scalar1=1.0 / 10000.0,
                                    scalar2=None, op0=Alu.mult)
            nc.scalar.activation(out=std0, in_=var, func=Act.Sqrt)
            # one Newton step: std = max(0.5*(std0 + var/std0), 1e-8)
            nc.vector.reciprocal(out=rs0, in_=std0)
            nc.vector.tensor_tensor(out=tmp, in0=var, in1=rs0, op=Alu.mult)
            nc.vector.tensor_tensor(out=tmp, in0=tmp, in1=std0, op=Alu.add)
            nc.vector.tensor_scalar(out=tmp, in0=tmp, scalar1=0.5, scalar2=1e-8,
                                    op0=Alu.mult, op1=Alu.max)
            # w = fw / std ; w2 = w*w
            FWt = singles.tile([1, F], f32)
            nc.sync.dma_start(out=FWt, in_=fw_ext[:])
            nc.vector.reciprocal(out=tmp2, in_=tmp)
            nc.vector.tensor_tensor(out=wv, in0=FWt, in1=tmp2, op=Alu.mult)
            nc.vector.tensor_tensor(out=tmp2, in0=wv, in1=wv, op=Alu.mult)
            nc.sync.dma_start(out=w2_bounce[:], in_=tmp2)
            w2bc = []
            for f in range(F):
                t = singles.tile([128, 1], f32, tag=f"w2bc{f}")
                w2bc.append(t)
                nc.sync.dma_start(out=t,
                                  in_=dram_ap(w2_bounce, f, [[0, 128], [1, 1]]))

            # ---------------- matmul operands (host-prebuilt fp16) -----------
            # triggers split across SP and ACT HWDGE queues: the trigger
            # instruction occupies its engine ~0.7us, so one engine would
            # serialize the whole prologue
            lhsT = []
            rhs = []
            for f in range(F):
                lt = singles.tile([KROWS, CP], f16, tag=f"lhsT{f}")
                rh = singles.tile([KROWS, HL], f16, tag=f"rhs{f}")
                lhsT.append(lt)
                rhs.append(rh)
                nc.sync.dma_start(out=lt, in_=lhs_ext[f])
                nc.sync.dma_start(out=rh, in_=rhs_ext[f])

            # ---------------- main dissimilarity + f-sum + stage 1 ----------
            psum_pool = tc.alloc_tile_pool(name="psum", bufs=2, space="PSUM")
            NEG_A = singles.tile([128, 4 * HL], f32)
            NEG_B = singles.tile([128, 4 * HL], f32)
            NEG_C = singles.tile([128, 4 * HL], f32)
            MX = singles.tile([128, 4, K1], f32)
            PX = singles.tile([128, 4, K1], u32)
            deferred_idx = []
            for cc in range(4):
                # one 3-bank psum tile + one wide ACT call per feature: the
                # ACT 352-cycle startup amortizes over the whole 1250 columns
                S = work.tile([128, F, HL], f32, tag="S")
                for f in range(F):
                    pt = psum_pool.tile([128, 1536], f32, tag="d2")
                    for (h0, hn) in HCH:
                        nc.tensor.matmul(
                            out=pt[:, h0:h0 + hn],
                            lhsT=lhsT[f][:, cc * 128:(cc + 1) * 128],
                            rhs=rhs[f][:, h0:h0 + hn],
                            start=True, stop=True,
                        )
                    nc.scalar.activation(out=S[:, f, :], in_=pt[:, 0:HL],
                                         func=Act.Sqrt, scale=w2bc[f])
                nc.gpsimd.tensor_tensor(out=S[:, 0:4, :], in0=S[:, 0:4, :],
                                        in1=S[:, 4:8, :], op=Alu.add)
                nc.vector.tensor_tensor(out=S[:, 0:2, :], in0=S[:, 0:2, :],
                                        in1=S[:, 2:4, :], op=Alu.add)
                nc.vector.tensor_tensor(out=S[:, 0, :], in0=S[:, 0, :],
                                        in1=S[:, 1, :], op=Alu.add)
                nc.vector.tensor_scalar(
                    out=NEG_A[:, cc * HL:(cc + 1) * HL],
                    in0=S[:, 0, :], scalar1=-1.0, scalar2=None, op0=Alu.mult)

                # stage-1 rounds: max + match_replace inline; max_index deferred
                # into the AllToAll window (A -> B -> C keeps each round's
                # input buffer intact for its deferred max_index)
                bufs = [NEG_A[:, cc * HL:(cc + 1) * HL],
                        NEG_B[:, cc * HL:(cc + 1) * HL],
                        NEG_C[:, cc * HL:(cc + 1) * HL]]
                for r in range(R1):
                    cur = bufs[r]
                    mx = MX[:, cc, 8 * r:8 * r + 8]
                    nc.vector.max(out=mx, in_=cur)
                    deferred_idx.append((PX[:, cc, 8 * r:8 * r + 8], mx, cur))
                    if r < R1 - 1:  # last round's replace output is never read
                        nc.vector.match_replace(out=bufs[r + 1], in_to_replace=mx,
                                                in_values=cur, imm_value=NEG_FILL)
            psum_pool.release()

            # ---------------- A2A (2 packing DMAs: one per 64-row half) -----
            for h in range(2):
                nc.sync.dma_start(
                    out=dram_ap(a2a_in, h * 64 * K1,
                                [[K1, 64], [2 * 64 * K1, 4], [1, K1]]),
                    in_=MX[64 * h:64 * h + 64, :, :],
                )
            nc.gpsimd.collective_compute(
                "AllToAll",
                mybir.AluOpType.bypass,
                replica_groups=[list(range(N_CORES))],
                ins=[a2a_in[:]],
                outs=[a2a_out[:]],
            )
            # deferred local max_index passes overlap the collective
            for px, mx, cur in deferred_idx:
                nc.vector.max_index(out=px, in_max=mx, in_values=cur)
            CANDA = singles.tile([64, N_CORES * K1], f32)
            CANDB = singles.tile([64, N_CORES * K1], f32)
            nc.sync.dma_start(
                out=CANDA,
                in_=dram_ap(a2a_out, 0, [[K1, 64], [64 * K1, N_CORES], [1, K1]]),
            )
            # ship original candidates before the rounds clobber CANDA
            nc.sync.dma_start(out=cand_ext[:], in_=CANDA[:])

            # ---------------- stage 2 ----------------
            GV = singles.tile([64, K2], f32)
            GP = singles.tile([64, K2], u32)
            bufs2 = [CANDA[:], CANDB[:]]
            for r in range(R2):
                cur = bufs2[r % 2]
                nxt = bufs2[(r + 1) % 2]
                gv = GV[:, 8 * r:8 * r + 8]
                nc.vector.max(out=gv, in_=cur)
                nc.vector.max_index(out=GP[:, 8 * r:8 * r + 8], in_max=gv, in_values=cur)
                if r < R2 - 1:
                    nc.vector.match_replace(out=nxt, in_to_replace=gv, in_values=cur,
                                            imm_value=NEG_FILL)

            # ---------------- outputs ----------------
            nc.sync.dma_start(
                out=dram_ap(pos_ext, 0, [[K1, 128], [128 * K1, 4], [1, K1]]),
                in_=PX[:],
            )
            nc.sync.dma_start(out=gpos_ext[:], in_=GP[:])
            nc.sync.dma_start(out=gval_ext[:], in_=GV[:])

    nc.compile()
    return nc


def _split16(x):
    """Exact fp32 = hi + lo decomposition into two fp16 parts (to ~2^-24)."""
    h = x.astype(np.float16)
    l = (x - h.astype(np.float32)).astype(np.float16)
    return h, l


def _prepare_inputs(x_t, x_h, y_h, feature_weights):
    xtT = x_t.T.astype(np.float32)                                # (8, 512)
    xhT = x_h.T.astype(np.float32)                                # (8, 10000)
    fw = feature_weights.reshape(1, F).astype(np.float32)

    xhf = np.zeros((F, NH_PAD), np.float32)
    xhf[:, :10000] = xhT

    # K rows (lhs, rhs) pairs per feature:
    #  0-2: (-2a_hi[w], b_hi[w])   3-5: (-2a_hi[w], b_lo[w])
    #  6-8: (-2a_lo[w], b_hi[w])   9: (q_hi, 1)  10: (q_lo, 1)
    #  11: (1, r_hi)  12: (1, r_lo)
    a = np.zeros((F, W, CP), np.float32)
    for w in range(W):
        a[:, w, :CP - w] = xtT[:, w:CP]
    q = ((a[:, 0] * a[:, 0] + a[:, 1] * a[:, 1]).astype(np.float32)
         + a[:, 2] * a[:, 2]).astype(np.float32)
    q[:, C:] = Q_PAD
    m2a = (-2.0 * a).astype(np.float32)
    m2a_h, m2a_l = _split16(m2a)
    q_h, q_l = _split16(q)
    lhs = np.zeros((F, KROWS, CP), np.float16)
    lhs[:, 0:W, :] = m2a_h
    lhs[:, W:2 * W, :] = m2a_h
    lhs[:, 2 * W:3 * W, :] = m2a_l
    lhs[:, 9, :] = q_h
    lhs[:, 10, :] = q_l
    lhs[:, 11:13, :] = 1.0

    in_maps = []
    for m in range(N_CORES):
        h0 = m * HL
        b = np.full((F, W, HL), H_PAD, np.float32)
        for w in range(W):
            n = max(0, min(HL, 10000 - (h0 + w)))
            b[:, w, :n] = xhT[:, h0 + w:h0 + w + n]
        r = ((b[:, 0] * b[:, 0] + b[:, 1] * b[:, 1]).astype(np.float32)
             + b[:, 2] * b[:, 2]).astype(np.float32)
        b_h, b_l = _split16(b)
        r_h, r_l = _split16(r)
        rhsd = np.zeros((F, KROWS, HL), np.float16)
        rhsd[:, 0:W, :] = b_h
        rhsd[:, W:2 * W, :] = b_l
        rhsd[:, 2 * W:3 * W, :] = b_h
        rhsd[:, 9:11, :] = 1.0
        rhsd[:, 11, :] = r_h
        rhsd[:, 12, :] = r_l
        in_maps.append({
            "lhs": lhs,
            "rhsd": rhsd,
            "xhf": xhf,
            "fw": fw,
        })
    return in_maps


def _assemble(results, y_h):
    """Host-side index chasing + y lookup. Returns (out, ok)."""
    yfull = y_h[:, 0].astype(np.float32)
    out = np.zeros((C, K), np.float32)
    ok = True
    pos_local = [results[m]["pos_local"] for m in range(N_CORES)]
    for owner in range(N_CORES):
        gp = results[owner]["gpos"]          # (64, 56) uint32
        gv = results[owner]["gval"]          # (64, 56)
        cand = results[owner]["cand"]        # (64, 192)
        nrows = 64 if owner < N_CORES - 1 else 64 - (CP - C)
        # validity: no core's 24th-best may beat the global 50th best
        last = cand[:nrows, K1 - 1::K1]
        if (last.max(axis=1) >= gv[:nrows, K - 1]).any():
            ok = False
        for j in range(nrows):
            c = owner * 64 + j
            p = gp[j, :K].astype(np.int64)
            m = p // K1
            t = p % K1
            lp = np.array([pos_local[mm][c, tt] for mm, tt in zip(m, t)], np.int64)
            out[c] = yfull[1 + HL * m + lp]
    return out, ok


def _reference_fallback(x_t, x_h, y_h, feature_weights):
    """Exact numpy replica of the reference (used only if the validity flag
    trips, which has probability ~1e-12)."""
    stds = np.maximum(np.std(x_h.astype(np.float32), axis=0), 1e-8)
    wn = (feature_weights.astype(np.float32) / stds).astype(np.float32)
    n_c = x_t.shape[0] - W + 1
    n_h = x_h.shape[0] - W + 1
    idx_c = np.arange(n_c)[:, None] + np.arange(W)[None, :]
    idx_h = np.arange(n_h)[:, None] + np.arange(W)[None, :]
    cw = x_t[idx_c]
    hw = x_h[idx_h]
    D = np.zeros((n_c, n_h), np.float32)
    for f in range(F):
        d2 = np.zeros((n_c, n_h), np.float32)
        for wi in range(W):
            diff = (cw[:, None, wi, f] - hw[None, :, wi, f]).astype(np.float32)
            d2 = (d2 + diff * diff).astype(np.float32)
        D = (D + wn[f] * np.sqrt(d2)).astype(np.float32)
    yv = y_h[W // 2:-(W // 2), 0]
    out = np.empty((n_c, K), np.float32)
    for c in range(n_c):
        order = np.lexsort((np.arange(n_h), D[c]))[:K]
        out[c] = yv[order]
    return out


def _ensure_axon_hooks():
    """The agent image's antenv lacks axon_hooks; inject an equivalent module
    and register the ctypes NTFF profile hook so trace=True works."""
    import sys
    import types
    import ctypes
    import contextlib

    try:
        from antenv.axon_hooks import get_axon_ntff_profile_hook  # noqa: F401
        return True
    except ImportError:
        pass
    try:
        import antenv
    except ImportError:
        return False
    so_path = "/opt/axon/libaxon_pjrt.so"
    if not os.path.exists(so_path):
        return False

    mod = types.ModuleType("antenv.axon_hooks")
    holder = {"hook": None}
    mod.set_axon_ntff_profile_hook = lambda h: holder.__setitem__("hook", h)
    mod.get_axon_ntff_profile_hook = lambda: holder["hook"]
    sys.modules["antenv.axon_hooks"] = mod
    antenv.axon_hooks = mod

    lib = ctypes.CDLL(so_path)
    if not hasattr(lib, "axon_start_nrt_profile"):
        return False
    lib.axon_start_nrt_profile.argtypes = [ctypes.POINTER(ctypes.c_int64),
                                           ctypes.c_size_t]
    lib.axon_start_nrt_profile.restype = ctypes.c_int64
    lib.axon_stop_nrt_profile.argtypes = [ctypes.c_char_p]
    lib.axon_stop_nrt_profile.restype = ctypes.c_int64

    @contextlib.contextmanager
    def _hook(output_dir, device_ids):
        import jax
        jax.devices()
        if device_ids:
            ids = (ctypes.c_int64 * len(device_ids))(*device_ids)
            rc = lib.axon_start_nrt_profile(ids, len(device_ids))
        else:
            rc = lib.axon_start_nrt_profile(None, 0)
        if rc != 0:
            raise RuntimeError(f"axon_start_nrt_profile rc={rc}")
        try:
            yield
        finally:
            n = lib.axon_stop_nrt_profile(str(output_dir).encode())
            if n <= 0:
                print(f"profile: {n} file(s) written to {output_dir}")

    mod.set_axon_ntff_profile_hook(_hook)
    return True


def kernel(x_t, x_h, y_h, feature_weights):
    global _GRAPH, LAST_EXEC_TIME_NS
    from concourse.bass_utils import run_bass_kernel_spmd

    x_t = np.asarray(x_t, np.float32)
    x_h = np.asarray(x_h, np.float32)
    y_h = np.asarray(y_h, np.float32)
    feature_weights = np.asarray(feature_weights, np.float32)

    if _GRAPH is None:
        _GRAPH = _build_graph()
    in_maps = _prepare_inputs(x_t, x_h, y_h, feature_weights)
    trace = bool(int(os.environ.get("KERNEL_TRACE", "0")))
    if trace and not _ensure_axon_hooks():
        trace = False
    res = run_bass_kernel_spmd(_GRAPH, in_maps, core_ids=list(range(N_CORES)),
                               trace=trace)
    LAST_EXEC_TIME_NS = res.exec_time_ns
    out, ok = _assemble(res.results, y_h)
    if not ok:
        out = _reference_fallback(x_t, x_h, y_h, feature_weights)
    return out


# revision 59
# speedup vs baseline: 1.2143x; 1.0369x over previous
"""Distributed AnEn (analog ensemble) kNN kernel for 8 TRN2 NeuronCores.

Strategy (SPMD, one graph on all 8 cores):
  - Historical axis sharded: core m owns dissimilarity columns [1250m, 1250m+1250).
  - Per-feature d2 = q - 2ab + r computed on the PE from host-prebuilt K=5
    operands (3 window rows, q row, ones row). The feature weight
    w_f = feature_weights/std(x_h) is computed on device from the full x_h and
    applied inside the ACT sqrt as a per-partition scale:
    sqrt(d2 * w_f^2) = w_f * sqrt(d2).
  - Feature-sum tree on gpsimd+DVE produces the negated dissimilarity slab.
  - Stage 1: 3 rounds of DVE max8/max_index/match_replace -> local top-24
    (sorted values + slab positions). 24 >= any core's share of the global
    top-50 with probability 1 - ~1e-12; a host-checked validity flag triggers
    an exact numpy fallback for the impossible miss.
  - AllToAll exchanges candidate values row-sharded (64 query rows per core);
    stage 2: 7 max8 rounds over the 192 gathered candidates -> global top-50
    in rank order. Tie-breaking matches jax.lax.top_k exactly (equal values
    resolve to the lower global index).
  - Host does pure index chasing + y lookup; every ordering decision is made
    on device.
"""

import os
import numpy as np

C = 510
CP = 512
HG = 9998
HL = 1250
F = 8
W = 3
K = 50
R1 = 2          # local rounds per 625-column half-shard -> 16 candidates/shard
R2 = 7          # global rounds -> 56 >= 50
K1 = 2 * 8 * R1  # 32 candidates/core (2 half-shards x 16)
K2 = 8 * R2
HLH = HL // 2   # 625
NEG_FILL = -1.0e30
Q_PAD = 30000.0  # q for padded query rows 510/511 (fp16-representable)
H_PAD = 100.0    # x_h pad value for history rows >= 9998+2 (r_pad = 30000)
KROWS = 13       # fp16 hi/lo decomposition rows per feature
N_CORES = 8
NH_TILES = 79   # 128 * 79 = 10112 padded x_h rows
NH_PAD = 128 * NH_TILES

_GRAPH = None
LAST_EXEC_TIME_NS = None


def _build_graph():
    import concourse.bass as bass
    import concourse.bacc as bacc
    import concourse.mybir as mybir
    import concourse.tile as tile

    f32 = mybir.dt.float32
    u32 = mybir.dt.uint32
    Alu = mybir.AluOpType
    Act = mybir.ActivationFunctionType

    nc = bacc.Bacc("TRN2", target_bir_lowering=False, debug=False,
                   num_devices=N_CORES)

    f16 = mybir.dt.float16
    lhs_ext = nc.declare_dram_parameter("lhs", [F, KROWS, CP], f16, False)
    rhs_ext = nc.declare_dram_parameter("rhsd", [F, KROWS, HL], f16, False)
    xhf_ext = nc.declare_dram_parameter("xhf", [F, NH_PAD], f32, False)
    fw_ext = nc.declare_dram_parameter("fw", [1, F], f32, False)
    pos_ext = nc.declare_dram_parameter("pos_local", [CP, K1], u32, True)
    cand_ext = nc.declare_dram_parameter("cand", [64, N_CORES * K1], f32, True)
    gpos_ext = nc.declare_dram_parameter("gpos", [64, K2], u32, True)
    gval_ext = nc.declare_dram_parameter("gval", [64, K2], f32, True)

    a2a_in = nc.dram_tensor("a2a_in", [N_CORES, 64, K1], f32)
    a2a_out = nc.dram_tensor("a2a_out", [N_CORES, 64, K1], f32)
    w2_bounce = nc.dram_tensor("w2_bounce", [F], f32)

    HCH = [(0, 512), (512, 512), (1024, 226)]  # h chunks within the 1250 slab

    def dram_ap(handle, offset, ap):
        full = handle[:]
        return bass.AP(tensor=full.tensor, offset=offset, ap=ap)

    with tile.TileContext(nc) as tc:
        with (
            tc.tile_pool(name="singles", bufs=1) as singles,
            tc.tile_pool(name="work", bufs=2) as work,
        ):
            # ---------------- stds from full x_h ----------------
            # X1[p, f, j] = padded x_h[79 p + j, f] (zero pads beyond 10000)
            X1 = singles.tile([128, F, NH_TILES], f32)
            nc.sync.dma_start(
                out=X1,
                in_=dram_ap(xhf_ext, 0,
                            [[NH_TILES, 128], [NH_PAD, F], [1, NH_TILES]]),
            )
            XQ = singles.tile([128, F, NH_TILES], f32)
            nc.vector.tensor_tensor(out=XQ, in0=X1, in1=X1, op=Alu.mult)

            ones128 = singles.tile([128, 1], f32)
            nc.vector.memset(ones128, 1.0)

            with tc.tile_pool(name="psum1", bufs=1, space="PSUM") as psum1_pool:
                ps_sum = psum1_pool.tile([1, NH_TILES * F], f32, tag="stats")
                SM = singles.tile([1, NH_TILES * F], f32)
                SM2 = singles.tile([1, NH_TILES * F], f32)
                x1flat = X1[:].rearrange("p f g -> p (f g)")
                xqflat = XQ[:].rearrange("p f g -> p (f g)")
                nc.tensor.matmul(out=ps_sum[:, 0:512], lhsT=ones128,
                                 rhs=x1flat[:, 0:512], start=True, stop=True)
                nc.tensor.matmul(out=ps_sum[:, 512:632], lhsT=ones128,
                                 rhs=x1flat[:, 512:632], start=True, stop=True)
                nc.vector.tensor_copy(SM, ps_sum)
                nc.tensor.matmul(out=ps_sum[:, 0:512], lhsT=ones128,
                                 rhs=xqflat[:, 0:512], start=True, stop=True)
                nc.tensor.matmul(out=ps_sum[:, 512:632], lhsT=ones128,
                                 rhs=xqflat[:, 512:632], start=True, stop=True)
                nc.vector.tensor_copy(SM2, ps_sum)

            # reduce the 79 per-chunk sums per feature with strided DVE adds
            def tree_reduce(t):
                v = t[:].rearrange("o (f g) -> o f g", f=F)
                n = NH_TILES
                while n > 1:
                    h = n // 2
                    nc.vector.tensor_tensor(out=v[:, :, 0:h], in0=v[:, :, 0:h],
                                            in1=v[:, :, h:2 * h], op=Alu.add)
                    if n % 2:
                        nc.vector.tensor_tensor(
                            out=v[:, :, 0:1], in0=v[:, :, 0:1],
                            in1=v[:, :, n - 1:n], op=Alu.add)
                    n = h
                return v[:, :, 0:1]

            Ssum = tree_reduce(SM).rearrange("o f x -> o (f x)")
            S2sum = tree_reduce(SM2).rearrange("o f x -> o (f x)")

            # mean/var/std/w^2 on [1, F]
            stats = singles.tile([1, 8 * F], f32)
            mu = stats[:, 0:F]
            muS = stats[:, F:2 * F]
            var = stats[:, 2 * F:3 * F]
            std0 = stats[:, 3 * F:4 * F]
            rs0 = stats[:, 4 * F:5 * F]
            wv = stats[:, 5 * F:6 * F]
            tmp = stats[:, 6 * F:7 * F]
            tmp2 = stats[:, 7 * F:8 * F]
            nc.vector.tensor_scalar(out=mu, in0=Ssum, scalar1=1.0 / 10000.0,
                                    scalar2=None, op0=Alu.mult)
            nc.vector.tensor_tensor(out=muS, in0=mu, in1=Ssum, op=Alu.mult)
            nc.vector.tensor_tensor(out=var, in0=S2sum, in1=muS, op=Alu.subtract)
            nc.vector.tensor_scalar(out=var, in0=var, scalar1=1.0 / 10000.0,
                                    scalar2=None, op0=Alu.mult)
            nc.scalar.activation(out=std0, in_=var, func=Act.Sqrt)
            # one Newton step: std = max(0.5*(std0 + var/std0), 1e-8)
            nc.vector.reciprocal(out=rs0, in_=std0)
            nc.vector.tensor_tensor(out=tmp, in0=var, in1=rs0, op=Alu.mult)
            nc.vector.tensor_tensor(out=tmp, in0=tmp, in1=std0, op=Alu.add)
            nc.vector.tensor_scalar(out=tmp, in0=tmp, scalar1=0.5, scalar2=1e-8,
                                    op0=Alu.mult, op1=Alu.max)
            # w = fw / std ; w2 = w*w
            FWt = singles.tile([1, F], f32)
            nc.sync.dma_start(out=FWt, in_=fw_ext[:])
            nc.vector.reciprocal(out=tmp2, in_=tmp)
            nc.vector.tensor_tensor(out=wv, in0=FWt, in1=tmp2, op=Alu.mult)
            nc.vector.tensor_tensor(out=tmp2, in0=wv, in1=wv, op=Alu.mult)
            nc.sync.dma_start(out=w2_bounce[:], in_=tmp2)
            w2bc = []
            for f in range(F):
                t = singles.tile([128, 1], f32, tag=f"w2bc{f}")
                w2bc.append(t)
                nc.sync.dma_start(out=t,
                                  in_=dram_ap(w2_bounce, f, [[0, 128], [1, 1]]))

            # ---------------- matmul operands (host-prebuilt fp16) -----------
            # triggers split across SP and ACT HWDGE queues: the trigger
            # instruction occupies its engine ~0.7us, so one engine would
            # serialize the whole prologue
            lhsT = []
            rhs = []
            for f in range(F):
                lt = singles.tile([KROWS, CP], f16, tag=f"lhsT{f}")
                rh = singles.tile([KROWS, HL], f16, tag=f"rhs{f}")
                lhsT.append(lt)
                rhs.append(rh)
                nc.sync.dma_start(out=lt, in_=lhs_ext[f])
                nc.sync.dma_start(out=rh, in_=rhs_ext[f])

            # ---------------- main dissimilarity + f-sum + stage 1 ----------
            psum_pool = tc.alloc_tile_pool(name="psum", bufs=2, space="PSUM")
            NEG_A = singles.tile([128, 4 * HL], f32)
            NEG_B = singles.tile([128, 4 * HL], f32)
            MX = singles.tile([128, 4, K1], f32)
            PX = singles.tile([128, 4, K1], u32)
            deferred_idx = []
            for cc in range(4):
                # one 3-bank psum tile + one wide ACT call per feature: the
                # ACT 352-cycle startup amortizes over the whole 1250 columns
                S = work.tile([128, F, HL], f32, tag="S")
                for f in range(F):
                    pt = psum_pool.tile([128, 1536], f32, tag="d2")
                    for (h0, hn) in HCH:
                        nc.tensor.matmul(
                            out=pt[:, h0:h0 + hn],
                            lhsT=lhsT[f][:, cc * 128:(cc + 1) * 128],
                            rhs=rhs[f][:, h0:h0 + hn],
                            start=True, stop=True,
                        )
                    nc.scalar.activation(out=S[:, f, :], in_=pt[:, 0:HL],
                                         func=Act.Sqrt, scale=w2bc[f])
                nc.gpsimd.tensor_tensor(out=S[:, 0:4, :], in0=S[:, 0:4, :],
                                        in1=S[:, 4:8, :], op=Alu.add)
                nc.vector.tensor_tensor(out=S[:, 0:2, :], in0=S[:, 0:2, :],
                                        in1=S[:, 2:4, :], op=Alu.add)
                nc.vector.tensor_tensor(out=S[:, 0, :], in0=S[:, 0, :],
                                        in1=S[:, 1, :], op=Alu.add)
                nc.vector.tensor_scalar(
                    out=NEG_A[:, cc * HL:(cc + 1) * HL],
                    in0=S[:, 0, :], scalar1=-1.0, scalar2=None, op0=Alu.mult)

                # stage-1: top-16 of each 625-column half-shard (2 max8 rounds);
                # max_index deferred into the AllToAll window (A -> B keeps each
                # round's input intact for its deferred max_index)
                for half in range(2):
                    base = cc * HL + half * HLH
                    bufs = [NEG_A[:, base:base + HLH], NEG_B[:, base:base + HLH]]
                    for r in range(R1):
                        cur = bufs[r]
                        s0 = half * 16 + 8 * r
                        mx = MX[:, cc, s0:s0 + 8]
                        nc.vector.max(out=mx, in_=cur)
                        deferred_idx.append((PX[:, cc, s0:s0 + 8], mx, cur))
                        if r < R1 - 1:  # last round's replace output never read
                            nc.vector.match_replace(out=bufs[r + 1], in_to_replace=mx,
                                                    in_values=cur,
                                                    imm_value=NEG_FILL)
            psum_pool.release()

            # ---------------- A2A (2 packing DMAs: one per 64-row half) -----
            for h in range(2):
                nc.sync.dma_start(
                    out=dram_ap(a2a_in, h * 64 * K1,
                                [[K1, 64], [2 * 64 * K1, 4], [1, K1]]),
                    in_=MX[64 * h:64 * h + 64, :, :],
                )
            nc.gpsimd.collective_compute(
                "AllToAll",
                mybir.AluOpType.bypass,
                replica_groups=[list(range(N_CORES))],
                ins=[a2a_in[:]],
                outs=[a2a_out[:]],
            )
            # deferred local max_index passes overlap the collective
            for px, mx, cur in deferred_idx:
                nc.vector.max_index(out=px, in_max=mx, in_values=cur)
            CANDA = singles.tile([64, N_CORES * K1], f32)
            CANDB = singles.tile([64, N_CORES * K1], f32)
            nc.sync.dma_start(
                out=CANDA,
                in_=dram_ap(a2a_out, 0, [[K1, 64], [64 * K1, N_CORES], [1, K1]]),
            )
            # ship original candidates before the rounds clobber CANDA
            nc.sync.dma_start(out=cand_ext[:], in_=CANDA[:])

            # ---------------- stage 2 ----------------
            GV = singles.tile([64, K2], f32)
            GP = singles.tile([64, K2], u32)
            bufs2 = [CANDA[:], CANDB[:]]
            for r in range(R2):
                cur = bufs2[r % 2]
                nxt = bufs2[(r + 1) % 2]
                gv = GV[:, 8 * r:8 * r + 8]
                nc.vector.max(out=gv, in_=cur)
                nc.vector.max_index(out=GP[:, 8 * r:8 * r + 8], in_max=gv, in_values=cur)
                if r < R2 - 1:
                    nc.vector.match_replace(out=nxt, in_to_replace=gv, in_values=cur,
                                            imm_value=NEG_FILL)

            # ---------------- outputs ----------------
            nc.sync.dma_start(
                out=dram_ap(pos_ext, 0, [[K1, 128], [128 * K1, 4], [1, K1]]),
                in_=PX[:],
            )
            nc.sync.dma_start(out=gpos_ext[:], in_=GP[:])
            nc.sync.dma_start(out=gval_ext[:], in_=GV[:])

    nc.compile()
    return nc


def _split16(x):
    """Exact fp32 = hi + lo decomposition into two fp16 parts (to ~2^-24)."""
    h = x.astype(np.float16)
    l = (x - h.astype(np.float32)).astype(np.float16)
    return h, l


def _prepare_inputs(x_t, x_h, y_h, feature_weights):
    xtT = x_t.T.astype(np.float32)                                # (8, 512)
    xhT = x_h.T.astype(np.float32)                                # (8, 10000)
    fw = feature_weights.reshape(1, F).astype(np.float32)

    xhf = np.zeros((F, NH_PAD), np.float32)
    xhf[:, :10000] = xhT

    # K rows (lhs, rhs) pairs per feature:
    #  0-2: (-2a_hi[w], b_hi[w])   3-5: (-2a_hi[w], b_lo[w])
    #  6-8: (-2a_lo[w], b_hi[w])   9: (q_hi, 1)  10: (q_lo, 1)
    #  11: (1, r_hi)  12: (1, r_lo)
    a = np.zeros((F, W, CP), np.float32)
    for w in range(W):
        a[:, w, :CP - w] = xtT[:, w:CP]
    q = ((a[:, 0] * a[:, 0] + a[:, 1] * a[:, 1]).astype(np.float32)
         + a[:, 2] * a[:, 2]).astype(np.float32)
    q[:, C:] = Q_PAD
    m2a = (-2.0 * a).astype(np.float32)
    m2a_h, m2a_l = _split16(m2a)
    q_h, q_l = _split16(q)
    lhs = np.zeros((F, KROWS, CP), np.float16)
    lhs[:, 0:W, :] = m2a_h
    lhs[:, W:2 * W, :] = m2a_h
    lhs[:, 2 * W:3 * W, :] = m2a_l
    lhs[:, 9, :] = q_h
    lhs[:, 10, :] = q_l
    lhs[:, 11:13, :] = 1.0

    in_maps = []
    for m in range(N_CORES):
        h0 = m * HL
        b = np.full((F, W, HL), H_PAD, np.float32)
        for w in range(W):
            n = max(0, min(HL, 10000 - (h0 + w)))
            b[:, w, :n] = xhT[:, h0 + w:h0 + w + n]
        r = ((b[:, 0] * b[:, 0] + b[:, 1] * b[:, 1]).astype(np.float32)
             + b[:, 2] * b[:, 2]).astype(np.float32)
        b_h, b_l = _split16(b)
        r_h, r_l = _split16(r)
        rhsd = np.zeros((F, KROWS, HL), np.float16)
        rhsd[:, 0:W, :] = b_h
        rhsd[:, W:2 * W, :] = b_l
        rhsd[:, 2 * W:3 * W, :] = b_h
        rhsd[:, 9:11, :] = 1.0
        rhsd[:, 11, :] = r_h
        rhsd[:, 12, :] = r_l
        in_maps.append({
            "lhs": lhs,
            "rhsd": rhsd,
            "xhf": xhf,
            "fw": fw,
        })
    return in_maps


def _assemble(results, y_h):
    """Host-side index chasing + y lookup. Returns (out, ok)."""
    yfull = y_h[:, 0].astype(np.float32)
    out = np.zeros((C, K), np.float32)
    ok = True
    pos_local = [results[m]["pos_local"] for m in range(N_CORES)]
    for owner in range(N_CORES):
        gp = results[owner]["gpos"]          # (64, 56) uint32
        gv = results[owner]["gval"]          # (64, 56)
        cand = results[owner]["cand"]        # (64, 192)
        nrows = 64 if owner < N_CORES - 1 else 64 - (CP - C)
        # validity: no half-shard's 16th-best may beat the global 50th best
        last = cand[:nrows, 15::16]
        if (last.max(axis=1) >= gv[:nrows, K - 1]).any():
            ok = False
        for j in range(nrows):
            c = owner * 64 + j
            p = gp[j, :K].astype(np.int64)
            m = p // K1
            rest = p % K1
            half = rest // 16
            lp = np.array([pos_local[mm][c, tt]
                           for mm, tt in zip(m, rest)], np.int64)
            out[c] = yfull[1 + HL * m + HLH * half + lp]
    return out, ok


def _reference_fallback(x_t, x_h, y_h, feature_weights):
    """Exact numpy replica of the reference (used only if the validity flag
    trips, which has probability ~1e-12)."""
    stds = np.maximum(np.std(x_h.astype(np.float32), axis=0), 1e-8)
    wn = (feature_weights.astype(np.float32) / stds).astype(np.float32)
    n_c = x_t.shape[0] - W + 1
    n_h = x_h.shape[0] - W + 1
    idx_c = np.arange(n_c)[:, None] + np.arange(W)[None, :]
    idx_h = np.arange(n_h)[:, None] + np.arange(W)[None, :]
    cw = x_t[idx_c]
    hw = x_h[idx_h]
    D = np.zeros((n_c, n_h), np.float32)
    for f in range(F):
        d2 = np.zeros((n_c, n_h), np.float32)
        for wi in range(W):
            diff = (cw[:, None, wi, f] - hw[None, :, wi, f]).astype(np.float32)
            d2 = (d2 + diff * diff).astype(np.float32)
        D = (D + wn[f] * np.sqrt(d2)).astype(np.float32)
    yv = y_h[W // 2:-(W // 2), 0]
    out = np.empty((n_c, K), np.float32)
    for c in range(n_c):
        order = np.lexsort((np.arange(n_h), D[c]))[:K]
        out[c] = yv[order]
    return out


def _ensure_axon_hooks():
    """The agent image's antenv lacks axon_hooks; inject an equivalent module
    and register the ctypes NTFF profile hook so trace=True works."""
    import sys
    import types
    import ctypes
    import contextlib

    try:
        from antenv.axon_hooks import get_axon_ntff_profile_hook  # noqa: F401
        return True
    except ImportError:
        pass
    try:
        import antenv
    except ImportError:
        return False
    so_path = "/opt/axon/libaxon_pjrt.so"
    if not os.path.exists(so_path):
        return False

    mod = types.ModuleType("antenv.axon_hooks")
    holder = {"hook": None}
    mod.set_axon_ntff_profile_hook = lambda h: holder.__setitem__("hook", h)
    mod.get_axon_ntff_profile_hook = lambda: holder["hook"]
    sys.modules["antenv.axon_hooks"] = mod
    antenv.axon_hooks = mod

    lib = ctypes.CDLL(so_path)
    if not hasattr(lib, "axon_start_nrt_profile"):
        return False
    lib.axon_start_nrt_profile.argtypes = [ctypes.POINTER(ctypes.c_int64),
                                           ctypes.c_size_t]
    lib.axon_start_nrt_profile.restype = ctypes.c_int64
    lib.axon_stop_nrt_profile.argtypes = [ctypes.c_char_p]
    lib.axon_stop_nrt_profile.restype = ctypes.c_int64

    @contextlib.contextmanager
    def _hook(output_dir, device_ids):
        import jax
        jax.devices()
        if device_ids:
            ids = (ctypes.c_int64 * len(device_ids))(*device_ids)
            rc = lib.axon_start_nrt_profile(ids, len(device_ids))
        else:
            rc = lib.axon_start_nrt_profile(None, 0)
        if rc != 0:
            raise RuntimeError(f"axon_start_nrt_profile rc={rc}")
        try:
            yield
        finally:
            n = lib.axon_stop_nrt_profile(str(output_dir).encode())
            if n <= 0:
                print(f"profile: {n} file(s) written to {output_dir}")

    mod.set_axon_ntff_profile_hook(_hook)
    return True


def kernel(x_t, x_h, y_h, feature_weights):
    global _GRAPH, LAST_EXEC_TIME_NS
    from concourse.bass_utils import run_bass_kernel_spmd

    x_t = np.asarray(x_t, np.float32)
    x_h = np.asarray(x_h, np.float32)
    y_h = np.asarray(y_h, np.float32)
    feature_weights = np.asarray(feature_weights, np.float32)

    if _GRAPH is None:
        _GRAPH = _build_graph()
    in_maps = _prepare_inputs(x_t, x_h, y_h, feature_weights)
    trace = bool(int(os.environ.get("KERNEL_TRACE", "0")))
    if trace and not _ensure_axon_hooks():
        trace = False
    res = run_bass_kernel_spmd(_GRAPH, in_maps, core_ids=list(range(N_CORES)),
                               trace=trace)
    LAST_EXEC_TIME_NS = res.exec_time_ns
    out, ok = _assemble(res.results, y_h)
    if not ok:
        out = _reference_fallback(x_t, x_h, y_h, feature_weights)
    return out
